# revision 9
# baseline (speedup 1.0000x reference)
"""Trainium2 Bass kernel for BatchChannelDecorrelationLoss.

Contract: kernel(**inputs) takes FULL unsharded inputs
  y:             (16, 192, 32, 32) f32
  x_hat:         (16, 3, 512, 512) f32
  target:        (16, 3, 512, 512) f32
  likelihoods_y: (16, 192, 32, 32) f32
and returns the FULL output: scalar f32 loss.

Strategy (data-parallel over batch N across 8 cores, 2 samples/core):
  device, per core:
    - per-(n,c) max / min of y over H*W (f32, exact)   -> stats (384, 2)
    - row-Gram B = Z^T Z over all 384 (n,c) rows, bf16 -> b0/b1/b2 tiles
      (upper block-triangle; host extracts the two per-sample 192x192
       diagonal blocks; bf16 is fine: corr term is ~1e-6 of the loss)
    - row sums via ones-vector matmul                  -> rs (1, 384)
    - (x_hat-target)^2 partial sums per partition      -> macc (128, 7)
    - sum(log(lik)) partial per partition              -> lnacc (128, 1)
  host:
    - rates = sum_n (round(max) - round(min))  [round commutes with max/min]
    - stable argsort -> top-64 channel idx  (matches jnp.argsort tie-break)
    - cov = (G_k - S_k S_k^T / M) / (M-1) on the selected 64x64 block
    - loss = lmbda*255^2*mse + bpp + lmbda_corr*sum(offdiag(cov)^2)

Engine/DMA choreography (engine streams execute in order, so program
order is placement):
  - Two balanced HWDGE load queues: sync = [y x3, xh chunks 0..6]
    (7.57 MB), scalar = [tg0..2, lik, tg3..6] (7.57 MB); chunk halves
    land nearly simultaneously so each subtract unblocks on time.
  - MSE chunk sizes decrease so the post-last-byte tail is tiny.
  - DVE stream: casts -> max/min reduces -> sub0..sub2 -> Gram PSUM
    copies -> sub3..sub6.  ACT stream: transpose copies + squares with
    Ln slotted into the idle gap around chunk 3.
  - All stores on the sync queue after its loads.
"""

import math
import sys

if "/opt/trn_rl_repo" not in sys.path:
    sys.path.insert(0, "/opt/trn_rl_repo")

import numpy as np

import concourse.bacc as bacc
import concourse.masks as masks
import concourse.mybir as mybir
import concourse.tile as tile
from concourse.bass_utils import run_bass_kernel_spmd

# ---- problem constants (hardcoded per spec) ----
N, C, HY, WY = 16, 192, 32, 32
NI, CI, HI, WI = 16, 3, 512, 512
TOP_K = 64
LMBDA = 0.01
LMBDA_CORR = 1e-4
N_CORES = 8
NS = N // N_CORES          # samples per core = 2
YROWS = NS * C             # 384
YCOLS = HY * WY            # 1024
MSE_COLS = NS * CI * HI * WI // 128   # 12288
LIK_COLS = NS * C * HY * WY // 128    # 3072
MSE_CHUNKS = [2560, 2560, 2048, 2048, 1536, 1024, 512]   # sums to 12288
N_MSE = len(MSE_CHUNKS)
NJ = YCOLS // 128                     # 8 hw chunks

FP32 = mybir.dt.float32
BF16 = mybir.dt.bfloat16
AX = mybir.AxisListType
OP = mybir.AluOpType
AF = mybir.ActivationFunctionType

_prog_cache = {}


def _build_program():
    nc = bacc.Bacc("TRN2", target_bir_lowering=False, debug=False,
                   num_devices=N_CORES)

    ys = nc.dram_tensor("ys", [YROWS, YCOLS], FP32, kind="ExternalInput")
    xh = nc.dram_tensor("xh", [128, MSE_COLS], FP32, kind="ExternalInput")
    tg = nc.dram_tensor("tg", [128, MSE_COLS], FP32, kind="ExternalInput")
    lk = nc.dram_tensor("lk", [128, LIK_COLS], FP32, kind="ExternalInput")

    stats = nc.dram_tensor("stats", [YROWS, 2], FP32, kind="ExternalOutput")
    b0 = nc.dram_tensor("b0", [128, 256], FP32, kind="ExternalOutput")
    b1 = nc.dram_tensor("b1", [128, 256], FP32, kind="ExternalOutput")
    b2 = nc.dram_tensor("b2", [128, 128], FP32, kind="ExternalOutput")
    rs = nc.dram_tensor("rs", [1, YROWS], FP32, kind="ExternalOutput")
    maccd = nc.dram_tensor("macc", [128, N_MSE], FP32, kind="ExternalOutput")
    lnd = nc.dram_tensor("lnacc", [128, 1], FP32, kind="ExternalOutput")

    chunk_off = [0]
    for w in MSE_CHUNKS:
        chunk_off.append(chunk_off[-1] + w)

    with tile.TileContext(nc) as tc:
        with (
            tc.tile_pool(name="singles", bufs=1) as singles,
            tc.tile_pool(name="ypool", bufs=3) as ypool,
            tc.tile_pool(name="ybf", bufs=3) as ybfp,
            tc.tile_pool(name="ztp", bufs=8) as ztp,
            tc.tile_pool(name="stp", bufs=3) as stp,
            tc.tile_pool(name="mx", bufs=1) as mxp,
            tc.tile_pool(name="mt", bufs=1) as mtp,
            tc.tile_pool(name="lkp", bufs=1) as lkp,
            tc.tile_pool(name="tpsum", bufs=4, space="PSUM") as tpsum,
            tc.tile_pool(name="gpsum", bufs=1, space="PSUM") as gpsum,
        ):
            # ---- loads: two balanced queues ----
            ytiles = []
            for t in range(3):
                yt = ypool.tile([128, YCOLS], FP32, tag="yt")
                nc.sync.dma_start(yt[:], ys[t * 128:(t + 1) * 128, :])
                ytiles.append(yt)

            mse_x = [mxp.tile([128, w], FP32, tag=f"xt{i}", name=f"xt{i}")
                     for i, w in enumerate(MSE_CHUNKS)]
            mse_t = [mtp.tile([128, w], FP32, tag=f"tt{i}", name=f"tt{i}")
                     for i, w in enumerate(MSE_CHUNKS)]
            for i in range(N_MSE):
                nc.sync.dma_start(mse_x[i][:],
                                  xh[:, chunk_off[i]:chunk_off[i + 1]])
            for i in range(3):
                nc.scalar.dma_start(mse_t[i][:],
                                    tg[:, chunk_off[i]:chunk_off[i + 1]])
            lt = lkp.tile([128, LIK_COLS], FP32)
            nc.scalar.dma_start(lt[:], lk[:])
            for i in range(3, N_MSE):
                nc.scalar.dma_start(mse_t[i][:],
                                    tg[:, chunk_off[i]:chunk_off[i + 1]])

            ident = singles.tile([128, 128], BF16)
            masks.make_identity(nc, ident[:])
            ones = singles.tile([128, 1], BF16)
            nc.gpsimd.memset(ones[:], 1.0)

            # ---- DVE: bf16 casts first (feed the PE chain) ----
            ybf = []
            for t in range(3):
                yb = ybfp.tile([128, YCOLS], BF16, tag="yb")
                nc.vector.tensor_copy(yb[:], ytiles[t][:])
                ybf.append(yb)

            # ---- PE transposes; ACT does the PSUM->SBUF copies ----
            zts = []
            for j in range(NJ):
                sl = slice(j * 128, (j + 1) * 128)
                zt = ztp.tile([128, YROWS], BF16, tag="zt")
                for t in range(3):
                    pt = tpsum.tile([128, 128], BF16, tag="tp")
                    nc.tensor.transpose(pt[:], ybf[t][:, sl], ident[:])
                    nc.scalar.copy(zt[:, t * 128:(t + 1) * 128], pt[:])
                zts.append(zt)

            # ---- DVE: per-(n,c) max/min; stores on sync ----
            sttiles = []
            for t in range(3):
                st = stp.tile([128, 2], FP32, tag="st")
                nc.vector.tensor_reduce(st[:, 0:1], ytiles[t][:], axis=AX.X,
                                        op=OP.max)
                nc.vector.tensor_reduce(st[:, 1:2], ytiles[t][:], axis=AX.X,
                                        op=OP.min)
                sttiles.append(st)

            macc = singles.tile([128, N_MSE], FP32)
            lnacc = singles.tile([128, 1], FP32)

            def mse_chunk(i):
                xt, tt = mse_x[i], mse_t[i]
                nc.vector.tensor_tensor(xt[:], xt[:], tt[:], op=OP.subtract)
                nc.scalar.activation(xt[:], xt[:], AF.Square,
                                     accum_out=macc[:, i:i + 1])

            mse_chunk(0)
            mse_chunk(1)

            # ---- row-Gram upper blocks + row sums, PSUM-accumulated ----
            pb0 = gpsum.tile([128, 256], FP32, tag="pb0")
            for j, zt in enumerate(zts):
                nc.tensor.matmul(pb0[:], lhsT=zt[:, 0:128], rhs=zt[:, 0:256],
                                 start=(j == 0), stop=(j == NJ - 1))
            pb1 = gpsum.tile([128, 256], FP32, tag="pb1")
            for j, zt in enumerate(zts):
                nc.tensor.matmul(pb1[:], lhsT=zt[:, 128:256],
                                 rhs=zt[:, 128:384],
                                 start=(j == 0), stop=(j == NJ - 1))
            pb2 = gpsum.tile([128, 128], FP32, tag="pb2")
            for j, zt in enumerate(zts):
                nc.tensor.matmul(pb2[:], lhsT=zt[:, 256:384],
                                 rhs=zt[:, 256:384],
                                 start=(j == 0), stop=(j == NJ - 1))
            prs = gpsum.tile([1, YROWS], FP32, tag="prs")
            for j, zt in enumerate(zts):
                nc.tensor.matmul(prs[:], lhsT=ones[:], rhs=zt[:],
                                 start=(j == 0), stop=(j == NJ - 1))

            mse_chunk(2)

            # ---- ACT: Ln in the gap around chunk 3 ----
            nc.scalar.activation(lt[:], lt[:], AF.Ln,
                                 accum_out=lnacc[:, 0:1])

            # ---- DVE: Gram PSUM -> SBUF while chunk 3 streams in ----
            gsb = []
            for psum_t, dram_t, w in ((pb0, b0, 256), (pb1, b1, 256),
                                      (pb2, b2, 128)):
                sb = singles.tile([128, w], FP32, tag=f"sb{w}",
                                  name=f"gout_{dram_t.name}")
                nc.vector.tensor_copy(sb[:], psum_t[:])
                gsb.append((sb, dram_t))
            rssb = singles.tile([1, YROWS], FP32)
            nc.vector.tensor_copy(rssb[:], prs[:])

            # ---- stores on sync (its loads are long issued) ----
            for t in range(3):
                nc.sync.dma_start(stats[t * 128:(t + 1) * 128, :],
                                  sttiles[t][:])
            for sb, dram_t in gsb:
                nc.sync.dma_start(dram_t[:], sb[:])
            nc.sync.dma_start(rs[:], rssb[:])
            nc.sync.dma_start(lnd[:], lnacc[:])

            for i in range(3, N_MSE):
                mse_chunk(i)

            nc.sync.dma_start(maccd[:], macc[:])

    nc.compile()
    return nc


def _get_program():
    if "nc" not in _prog_cache:
        _prog_cache["nc"] = _build_program()
    return _prog_cache["nc"]


def kernel(y, x_hat, target, likelihoods_y):
    y = np.ascontiguousarray(y, dtype=np.float32)
    x_hat = np.ascontiguousarray(x_hat, dtype=np.float32)
    target = np.ascontiguousarray(target, dtype=np.float32)
    lik = np.ascontiguousarray(likelihoods_y, dtype=np.float32)

    nc = _get_program()

    in_maps = []
    for c in range(N_CORES):
        s = slice(c * NS, (c + 1) * NS)
        in_maps.append({
            "ys": y[s].reshape(YROWS, YCOLS),
            "xh": x_hat[s].reshape(128, MSE_COLS),
            "tg": target[s].reshape(128, MSE_COLS),
            "lk": lik[s].reshape(128, LIK_COLS),
        })

    res = run_bass_kernel_spmd(nc, in_maps, list(range(N_CORES)))
    results = res.results

    # ---- host-side combine (all O(C^2) and smaller) ----
    stats = np.stack([r["stats"] for r in results])       # (8, 384, 2)
    stats = stats.reshape(N, C, 2)
    fmax, fmin = stats[..., 0], stats[..., 1]

    # rates: round commutes with max/min; np.round == jnp.round (half-to-even)
    per_sample = np.round(fmax).astype(np.int64) - np.round(fmin).astype(np.int64)
    rates = per_sample.sum(axis=0)                        # (192,)
    idx = np.argsort(rates, kind="stable")[::-1][:TOP_K]

    # row-Gram -> per-channel Gram G and sums S
    B = np.zeros((YROWS, YROWS), dtype=np.float64)
    for r in results:
        B[0:128, 0:256] += r["b0"]
        B[128:256, 128:384] += r["b1"]
        B[256:384, 256:384] += r["b2"]
    B = np.triu(B) + np.triu(B, 1).T
    G = B[0:C, 0:C] + B[C:2 * C, C:2 * C]

    rs_all = np.sum([r["rs"] for r in results], axis=0,
                    dtype=np.float64).reshape(YROWS)
    S = rs_all[0:C] + rs_all[C:2 * C]

    M = N * HY * WY                                       # 16384
    Gk = G[np.ix_(idx, idx)]
    Sk = S[idx]
    cov = (Gk - np.outer(Sk, Sk) / M) / (M - 1)
    off = cov - np.diag(np.diag(cov))
    corr_loss = float(np.sum(off ** 2))

    mse_sum = float(np.sum([r["macc"] for r in results], dtype=np.float64))
    ln_sum = float(np.sum([r["lnacc"] for r in results], dtype=np.float64))

    num_pixels = N * HI * WI
    mse_loss = mse_sum / (NI * CI * HI * WI)
    bpp_loss = ln_sum / (-math.log(2) * num_pixels)
    loss = LMBDA * 255.0 ** 2 * mse_loss + bpp_loss + LMBDA_CORR * corr_loss
    return np.float32(loss)


# revision 10
# speedup vs baseline: 1.1425x; 1.1425x over previous
"""Trainium2 Bass kernel for BatchChannelDecorrelationLoss.

Contract: kernel(**inputs) takes FULL unsharded inputs
  y:             (16, 192, 32, 32) f32
  x_hat:         (16, 3, 512, 512) f32
  target:        (16, 3, 512, 512) f32
  likelihoods_y: (16, 192, 32, 32) f32
and returns the FULL output: scalar f32 loss.

Strategy (data-parallel over batch N across 8 cores, 2 samples/core):
  device, per core:
    - per-(n,c) max / min of y over H*W (f32, exact)   -> stats (384, 2)
    - row-Gram B = Z^T Z over all 384 (n,c) rows, bf16 -> b0/b1/b2 tiles
      (upper block-triangle; host extracts the two per-sample 192x192
       diagonal blocks; bf16 is fine: corr term is ~1e-6 of the loss)
    - row sums via ones-vector matmul                  -> rs (1, 384)
    - (x_hat-target)^2 partial sums per partition      -> macc (128, 7)
    - sum(log(lik)) partial per partition              -> lnacc (128, 1)
  host:
    - rates = sum_n (round(max) - round(min))  [round commutes with max/min]
    - stable argsort -> top-64 channel idx  (matches jnp.argsort tie-break)
    - cov = (G_k - S_k S_k^T / M) / (M-1) on the selected 64x64 block
    - loss = lmbda*255^2*mse + bpp + lmbda_corr*sum(offdiag(cov)^2)

Engine/DMA choreography (engine streams execute in order, so program
order is placement):
  - Two balanced HWDGE load queues: sync = [y x3, xh chunks 0..6]
    (7.57 MB), scalar = [tg0..2, lik, tg3..6] (7.57 MB); chunk halves
    land nearly simultaneously so each subtract unblocks on time.
  - MSE chunk sizes decrease so the post-last-byte tail is tiny.
  - DVE stream: casts -> max/min reduces -> sub0..sub2 -> Gram PSUM
    copies -> sub3..sub6.  ACT stream: transpose copies + squares with
    Ln slotted into the idle gap around chunk 3.
  - All stores on the sync queue after its loads.
"""

import math
import sys

if "/opt/trn_rl_repo" not in sys.path:
    sys.path.insert(0, "/opt/trn_rl_repo")

import numpy as np

import concourse.bacc as bacc
import concourse.masks as masks
import concourse.mybir as mybir
import concourse.tile as tile
from concourse.bass_utils import run_bass_kernel_spmd

# ---- problem constants (hardcoded per spec) ----
N, C, HY, WY = 16, 192, 32, 32
NI, CI, HI, WI = 16, 3, 512, 512
TOP_K = 64
LMBDA = 0.01
LMBDA_CORR = 1e-4
N_CORES = 8
NS = N // N_CORES          # samples per core = 2
YROWS = NS * C             # 384
YCOLS = HY * WY            # 1024
MSE_COLS = NS * CI * HI * WI // 128   # 12288
LIK_COLS = NS * C * HY * WY // 128    # 3072
MSE_CHUNKS = [2560, 2560, 2048, 2048, 1536, 1024, 512]   # sums to 12288
N_MSE = len(MSE_CHUNKS)
NJ = YCOLS // 128                     # 8 hw chunks

FP32 = mybir.dt.float32
BF16 = mybir.dt.bfloat16
AX = mybir.AxisListType
OP = mybir.AluOpType
AF = mybir.ActivationFunctionType

_prog_cache = {}


def _build_program():
    nc = bacc.Bacc("TRN2", target_bir_lowering=False, debug=False,
                   num_devices=N_CORES)

    ys = nc.dram_tensor("ys", [YROWS, YCOLS], FP32, kind="ExternalInput")
    xh = nc.dram_tensor("xh", [128, MSE_COLS], FP32, kind="ExternalInput")
    tg = nc.dram_tensor("tg", [128, MSE_COLS], FP32, kind="ExternalInput")
    lk = nc.dram_tensor("lk", [128, LIK_COLS], FP32, kind="ExternalInput")

    stats = nc.dram_tensor("stats", [YROWS, 2], FP32, kind="ExternalOutput")
    b0 = nc.dram_tensor("b0", [128, 256], FP32, kind="ExternalOutput")
    b1 = nc.dram_tensor("b1", [128, 256], FP32, kind="ExternalOutput")
    b2 = nc.dram_tensor("b2", [128, 128], FP32, kind="ExternalOutput")
    rs = nc.dram_tensor("rs", [1, YROWS], FP32, kind="ExternalOutput")
    maccd = nc.dram_tensor("macc", [128, N_MSE], FP32, kind="ExternalOutput")
    lnd = nc.dram_tensor("lnacc", [128, 1], FP32, kind="ExternalOutput")

    chunk_off = [0]
    for w in MSE_CHUNKS:
        chunk_off.append(chunk_off[-1] + w)

    with tile.TileContext(nc) as tc:
        with (
            tc.tile_pool(name="singles", bufs=1) as singles,
            tc.tile_pool(name="ypool", bufs=3) as ypool,
            tc.tile_pool(name="ybf", bufs=3) as ybfp,
            tc.tile_pool(name="ztp", bufs=8) as ztp,
            tc.tile_pool(name="stp", bufs=3) as stp,
            tc.tile_pool(name="mx", bufs=1) as mxp,
            tc.tile_pool(name="mt", bufs=1) as mtp,
            tc.tile_pool(name="lkp", bufs=1) as lkp,
            tc.tile_pool(name="tpsum", bufs=4, space="PSUM") as tpsum,
            tc.tile_pool(name="gpsum", bufs=1, space="PSUM") as gpsum,
        ):
            # ---- loads ----
            # scalar queue: only early items (its ring waits resolve
            # before ACT compute); sync queue: everything else.
            ytiles = []
            for t in range(3):
                yt = ypool.tile([128, YCOLS], FP32, tag="yt")
                nc.sync.dma_start(yt[:], ys[t * 128:(t + 1) * 128, :])
                ytiles.append(yt)

            mse_x = [mxp.tile([128, w], FP32, tag=f"xt{i}", name=f"xt{i}")
                     for i, w in enumerate(MSE_CHUNKS)]
            mse_t = [mtp.tile([128, w], FP32, tag=f"tt{i}", name=f"tt{i}")
                     for i, w in enumerate(MSE_CHUNKS)]

            def load_x(i, eng):
                eng.dma_start(mse_x[i][:], xh[:, chunk_off[i]:chunk_off[i + 1]])

            def load_t(i, eng):
                eng.dma_start(mse_t[i][:], tg[:, chunk_off[i]:chunk_off[i + 1]])

            # scalar: xh0, tg0, tg1, tg2, lik
            load_x(0, nc.scalar)
            load_t(0, nc.scalar)
            load_t(1, nc.scalar)
            load_t(2, nc.scalar)
            lt = lkp.tile([128, LIK_COLS], FP32)
            nc.scalar.dma_start(lt[:], lk[:])
            # sync: xh1..3, then interleaved pairs
            load_x(1, nc.sync)
            load_x(2, nc.sync)
            load_x(3, nc.sync)
            load_t(3, nc.sync)
            load_x(4, nc.sync)
            load_t(4, nc.sync)
            load_x(5, nc.sync)
            load_t(5, nc.sync)
            load_x(6, nc.sync)
            load_t(6, nc.sync)

            ident = singles.tile([128, 128], BF16)
            masks.make_identity(nc, ident[:])
            ones = singles.tile([128, 1], BF16)
            nc.gpsimd.memset(ones[:], 1.0)

            # ---- DVE: bf16 casts first (feed the PE chain) ----
            ybf = []
            for t in range(3):
                yb = ybfp.tile([128, YCOLS], BF16, tag="yb")
                nc.vector.tensor_copy(yb[:], ytiles[t][:])
                ybf.append(yb)

            # ---- PE transposes into one PSUM tile per hw-chunk; one
            # DVE copy moves all 384 columns to SBUF ----
            zts = []
            zt_copies = []
            for j in range(NJ):
                sl = slice(j * 128, (j + 1) * 128)
                zt = ztp.tile([128, YROWS], BF16, tag="zt")
                pt = tpsum.tile([128, YROWS], BF16, tag="tp")
                for t in range(3):
                    nc.tensor.transpose(pt[:, t * 128:(t + 1) * 128],
                                        ybf[t][:, sl], ident[:])
                zt_copies.append((zt, pt))
                zts.append(zt)

            # ---- DVE: per-(n,c) max/min; stores on sync ----
            sttiles = []
            for t in range(3):
                st = stp.tile([128, 2], FP32, tag="st")
                nc.vector.tensor_reduce(st[:, 0:1], ytiles[t][:], axis=AX.X,
                                        op=OP.max)
                nc.vector.tensor_reduce(st[:, 1:2], ytiles[t][:], axis=AX.X,
                                        op=OP.min)
                sttiles.append(st)

            macc = singles.tile([128, N_MSE], FP32)
            lnacc = singles.tile([128, 1], FP32)

            def mse_chunk(i):
                xt, tt = mse_x[i], mse_t[i]
                nc.vector.tensor_tensor(xt[:], xt[:], tt[:], op=OP.subtract)
                nc.scalar.activation(xt[:], xt[:], AF.Square,
                                     accum_out=macc[:, i:i + 1])

            mse_chunk(0)
            for zt, pt in zt_copies:
                nc.vector.tensor_copy(zt[:], pt[:])
            mse_chunk(1)

            # ---- row-Gram upper blocks + row sums, PSUM-accumulated ----
            pb0 = gpsum.tile([128, 256], FP32, tag="pb0")
            for j, zt in enumerate(zts):
                nc.tensor.matmul(pb0[:], lhsT=zt[:, 0:128], rhs=zt[:, 0:256],
                                 start=(j == 0), stop=(j == NJ - 1))
            pb1 = gpsum.tile([128, 256], FP32, tag="pb1")
            for j, zt in enumerate(zts):
                nc.tensor.matmul(pb1[:], lhsT=zt[:, 128:256],
                                 rhs=zt[:, 128:384],
                                 start=(j == 0), stop=(j == NJ - 1))
            pb2 = gpsum.tile([128, 128], FP32, tag="pb2")
            for j, zt in enumerate(zts):
                nc.tensor.matmul(pb2[:], lhsT=zt[:, 256:384],
                                 rhs=zt[:, 256:384],
                                 start=(j == 0), stop=(j == NJ - 1))
            prs = gpsum.tile([1, YROWS], FP32, tag="prs")
            for j, zt in enumerate(zts):
                nc.tensor.matmul(prs[:], lhsT=ones[:], rhs=zt[:],
                                 start=(j == 0), stop=(j == NJ - 1))

            mse_chunk(2)

            # ---- ACT: Ln in the gap around chunk 3 ----
            nc.scalar.activation(lt[:], lt[:], AF.Ln,
                                 accum_out=lnacc[:, 0:1])

            # ---- DVE: Gram PSUM -> SBUF while chunk 3 streams in ----
            gsb = []
            for psum_t, dram_t, w in ((pb0, b0, 256), (pb1, b1, 256),
                                      (pb2, b2, 128)):
                sb = singles.tile([128, w], FP32, tag=f"sb{w}",
                                  name=f"gout_{dram_t.name}")
                nc.vector.tensor_copy(sb[:], psum_t[:])
                gsb.append((sb, dram_t))
            rssb = singles.tile([1, YROWS], FP32)
            nc.vector.tensor_copy(rssb[:], prs[:])

            # ---- stores on sync (its loads are long issued) ----
            for t in range(3):
                nc.sync.dma_start(stats[t * 128:(t + 1) * 128, :],
                                  sttiles[t][:])
            for sb, dram_t in gsb:
                nc.sync.dma_start(dram_t[:], sb[:])
            nc.sync.dma_start(rs[:], rssb[:])
            nc.sync.dma_start(lnd[:], lnacc[:])

            for i in range(3, N_MSE):
                mse_chunk(i)

            nc.scalar.dma_start(maccd[:], macc[:])

    nc.compile()
    return nc


def _get_program():
    if "nc" not in _prog_cache:
        _prog_cache["nc"] = _build_program()
    return _prog_cache["nc"]


def kernel(y, x_hat, target, likelihoods_y):
    y = np.ascontiguousarray(y, dtype=np.float32)
    x_hat = np.ascontiguousarray(x_hat, dtype=np.float32)
    target = np.ascontiguousarray(target, dtype=np.float32)
    lik = np.ascontiguousarray(likelihoods_y, dtype=np.float32)

    nc = _get_program()

    in_maps = []
    for c in range(N_CORES):
        s = slice(c * NS, (c + 1) * NS)
        in_maps.append({
            "ys": y[s].reshape(YROWS, YCOLS),
            "xh": x_hat[s].reshape(128, MSE_COLS),
            "tg": target[s].reshape(128, MSE_COLS),
            "lk": lik[s].reshape(128, LIK_COLS),
        })

    res = run_bass_kernel_spmd(nc, in_maps, list(range(N_CORES)))
    results = res.results

    # ---- host-side combine (all O(C^2) and smaller) ----
    stats = np.stack([r["stats"] for r in results])       # (8, 384, 2)
    stats = stats.reshape(N, C, 2)
    fmax, fmin = stats[..., 0], stats[..., 1]

    # rates: round commutes with max/min; np.round == jnp.round (half-to-even)
    per_sample = np.round(fmax).astype(np.int64) - np.round(fmin).astype(np.int64)
    rates = per_sample.sum(axis=0)                        # (192,)
    idx = np.argsort(rates, kind="stable")[::-1][:TOP_K]

    # row-Gram -> per-channel Gram G and sums S
    B = np.zeros((YROWS, YROWS), dtype=np.float64)
    for r in results:
        B[0:128, 0:256] += r["b0"]
        B[128:256, 128:384] += r["b1"]
        B[256:384, 256:384] += r["b2"]
    B = np.triu(B) + np.triu(B, 1).T
    G = B[0:C, 0:C] + B[C:2 * C, C:2 * C]

    rs_all = np.sum([r["rs"] for r in results], axis=0,
                    dtype=np.float64).reshape(YROWS)
    S = rs_all[0:C] + rs_all[C:2 * C]

    M = N * HY * WY                                       # 16384
    Gk = G[np.ix_(idx, idx)]
    Sk = S[idx]
    cov = (Gk - np.outer(Sk, Sk) / M) / (M - 1)
    off = cov - np.diag(np.diag(cov))
    corr_loss = float(np.sum(off ** 2))

    mse_sum = float(np.sum([r["macc"] for r in results], dtype=np.float64))
    ln_sum = float(np.sum([r["lnacc"] for r in results], dtype=np.float64))

    num_pixels = N * HI * WI
    mse_loss = mse_sum / (NI * CI * HI * WI)
    bpp_loss = ln_sum / (-math.log(2) * num_pixels)
    loss = LMBDA * 255.0 ** 2 * mse_loss + bpp_loss + LMBDA_CORR * corr_loss
    return np.float32(loss)
